# revision 10
# baseline (speedup 1.0000x reference)
"""Trainium2 Bass kernel for nn_GATNet: 3-layer GAT + global max pool + prototype head.

Sharding: nodes (and their in-edges) sharded across 8 NeuronCores; dense/update
phases local per shard with an AllGather of the projected feature table per
layer; prototype/MLP tail replicated on all cores.

Algorithm (validated vs reference in fp64/np): edges sorted by dst, grouped into
128-row aligned dst blocks; per edge-tile a one-hot selection matrix S is built
on DVE (is_equal vs iota) and the attention-weighted message aggregation is a
PE matmul S^T @ [Mw | u] accumulated in PSUM per block. Softmax normalization
is deferred to a per-row divide after aggregation (segment-max is skipped:
alpha is bounded by a few units, exp() cannot overflow in fp32).
"""
import os
import time
import numpy as np
import concourse.bass as bass
import concourse.tile as tile
from concourse import bacc, mybir
from concourse.bass_utils import run_bass_kernel_spmd
from concourse.masks import make_identity

F32, F16, I32, I16 = mybir.dt.float32, mybir.dt.float16, mybir.dt.int32, mybir.dt.int16

# problem constants (hardcoded per harness contract)
N = 50000
E = 800000
G = 64
HEADS = 4
HID = 25
DENSE = 100
IN_DIM = 128
OUT_DIM = 2
NUM_PROT = 10
EPS = 1e-4
NEG = 0.2

NC = 8
NPC = N // NC            # 6250 nodes per core
ROW = 128
NBLK = (NPC + ROW - 1) // ROW      # 49
NPAD = NBLK * ROW                  # 6272
TROWS = NC * NPAD                  # 50176 rows in gathered table
HALF = TROWS // 2                  # 25088 (int16-safe half tables)
CPAD = 128                         # hext row: [h(100) | es(4) | ed(4) | pad]
SENT = 1000.0                      # sentinel dst_rel for pad slots
PSLOT = 1024                       # pooling slots per graph
PCOLS = PSLOT // 128               # 8


def _wrap16(lst):
    """dma_gather idx layout: idx j -> partition j%16, col j//16; replicated x8."""
    ni = lst.shape[0]
    w = lst.reshape(ni // 16, 16).T.astype(np.int16)
    return np.tile(w, (8, 1))


def preprocess(edge_index, batch):
    """CPU sharding: per-core per-block edge slots (A/B half split), pooling maps."""
    src = np.concatenate([np.asarray(edge_index[0]), np.arange(N, dtype=np.int64)])
    dst = np.concatenate([np.asarray(edge_index[1]), np.arange(N, dtype=np.int64)])
    gpad = (src // NPC) * NPAD + (src % NPC)    # padded-global row of src
    core = dst // NPC

    per = []
    tpa = tpb = 0
    for k in range(NC):
        m = core == k
        s_k = gpad[m]
        d_loc = (dst[m] - k * NPC).astype(np.int64)
        order = np.argsort(d_loc, kind="stable")
        s_k, d_loc = s_k[order], d_loc[order]
        blk = d_loc // ROW
        # per block: split by src half
        lists = []
        for b in range(NBLK):
            sel = blk == b
            sb, db = s_k[sel], d_loc[sel]
            a = sb < HALF
            na, nb_ = int(a.sum()), int((~a).sum())
            lists.append((sb[a], db[a], sb[~a] - HALF, db[~a]))
            tpa = max(tpa, -(-na // 128))
            tpb = max(tpb, -(-nb_ // 128))
        per.append(lists)
    TPA, TPB_, C = tpa, tpb, tpa + tpb
    NIA, NIB = TPA * 128, TPB_ * 128

    cores = []
    for k in range(NC):
        midx = np.zeros((NBLK, 128, C * 8), np.int16)
        edidx = np.zeros((NBLK, 128, C * 8), np.int16)
        dstrel = np.zeros((NBLK, 128, C), np.float16)
        for b in range(NBLK):
            sa, da, sb, db = per[k][b]
            la = np.zeros(NIA, np.int64)
            lb = np.zeros(NIB, np.int64)
            la[:len(sa)] = sa
            lb[:len(sb)] = sb
            ea = np.zeros(NIA, np.int64)
            eb = np.zeros(NIB, np.int64)
            ea[:len(da)] = da
            eb[:len(db)] = db
            midx[b] = np.concatenate([_wrap16(la), _wrap16(lb)], axis=1)
            edidx[b] = np.concatenate([_wrap16(ea), _wrap16(eb)], axis=1)
            dr = np.full(C * 128, SENT, np.float32)
            dr[:len(da)] = da - b * ROW
            dr[NIA:NIA + len(db)] = db - b * ROW
            dstrel[b] = dr.reshape(C, 128).T.astype(np.float16)
        cores.append(dict(midx=midx, edidx=edidx, dstrel=dstrel))

    # pooling: per-core local-graph slot lists
    batch = np.asarray(batch)
    glp = 0
    pool = []
    for k in range(NC):
        bl = batch[k * NPC:(k + 1) * NPC]
        gs = np.unique(bl)
        glp = max(glp, len(gs))
        pool.append(gs)
    GLP = glp
    maxcnt = 0
    for k in range(NC):
        bl = batch[k * NPC:(k + 1) * NPC]
        for g in pool[k]:
            maxcnt = max(maxcnt, int((bl == g).sum()))
    pcols = max(PCOLS, -(-maxcnt // 128))
    pslot = pcols * 128
    for k in range(NC):
        bl = batch[k * NPC:(k + 1) * NPC]
        pidx = np.zeros((GLP, 128, pcols * 8), np.int16)
        gids = np.full((GLP, 1), G, np.int32)          # scratch row 64 for pads
        for j, g in enumerate(pool[k]):
            rows = np.nonzero(bl == g)[0]
            lst = np.full(pslot, NPC, np.int64)        # pad -> zero row 6250
            lst[:len(rows)] = rows
            pidx[j] = _wrap16(lst)
            gids[j, 0] = g
        cores[k]["pidx"] = pidx
        cores[k]["gids"] = gids
    return cores, TPA, TPB_, GLP, pcols


def build(TPA, TPB_, GLP, PC, repeat=1):
    """Build the SPMD Bass program (same for all 8 cores)."""
    C = TPA + TPB_
    NIA, NIB = TPA * 128, TPB_ * 128
    nc = bacc.Bacc("TRN2", target_bir_lowering=False, debug=False)

    # ---- I/O ----
    t_xT = nc.dram_tensor("xT", [IN_DIM, NPAD], F16, kind="ExternalInput")
    t_wext = [nc.dram_tensor(f"wext{i}", [128, 108], F16, kind="ExternalInput") for i in range(3)]
    t_bt = [nc.dram_tensor(f"bt{i}", [128, DENSE], F32, kind="ExternalInput") for i in range(3)]
    t_midx = nc.dram_tensor("midx", [NBLK, 128, C * 8], I16, kind="ExternalInput")
    t_edidx = nc.dram_tensor("edidx", [NBLK, 128, C * 8], I16, kind="ExternalInput")
    t_dstrel = nc.dram_tensor("dstrel", [NBLK, 128, C], F16, kind="ExternalInput")
    t_pidx = nc.dram_tensor("pidx", [GLP, 128, PC * 8], I16, kind="ExternalInput")
    t_gids = nc.dram_tensor("gids", [GLP, 1], I32, kind="ExternalInput")
    t_prhs = nc.dram_tensor("prhs", [128, NUM_PROT], F32, kind="ExternalInput")
    t_lwT = nc.dram_tensor("lwT", [NUM_PROT, OUT_DIM], F32, kind="ExternalInput")

    o_node = nc.dram_tensor("o_node", [NPAD, DENSE], F32, kind="ExternalOutput")
    o_logits = nc.dram_tensor("o_logits", [G, OUT_DIM], F32, kind="ExternalOutput")
    o_probs = nc.dram_tensor("o_probs", [G, OUT_DIM], F32, kind="ExternalOutput")
    o_gemb = nc.dram_tensor("o_gemb", [G, DENSE], F32, kind="ExternalOutput")
    o_dist = nc.dram_tensor("o_dist", [G, NUM_PROT], F32, kind="ExternalOutput")

    # ---- internal DRAM ----
    d_hextloc = nc.dram_tensor("hextloc", [NPAD, CPAD], F16)
    d_hexfull = nc.dram_tensor("hexfull", [TROWS, CPAD], F16, addr_space="Shared")
    d_edtab = nc.dram_tensor("edtab", [NPAD, 128], F16)
    d_h3tab = nc.dram_tensor("h3tab", [NPAD + 128, 128], F16)
    d_canvas = nc.dram_tensor("canvas", [G + 1, DENSE], F32)
    d_gout = nc.dram_tensor("gout", [G, DENSE], F32, addr_space="Shared")

    with tile.TileContext(nc) as tc:
        import contextlib
        ctx = contextlib.ExitStack()
        const = ctx.enter_context(tc.tile_pool(name="const", bufs=1))
        slabp = ctx.enter_context(tc.tile_pool(name="slab", bufs=1))
        wp = ctx.enter_context(tc.tile_pool(name="wp", bufs=1))
        iop = ctx.enter_context(tc.tile_pool(name="io", bufs=3))
        big = ctx.enter_context(tc.tile_pool(name="big", bufs=2))
        sm = ctx.enter_context(tc.tile_pool(name="sm", bufs=2))
        psum = ctx.enter_context(tc.tile_pool(name="ps", bufs=1, space="PSUM"))
        psa = ctx.enter_context(tc.tile_pool(name="psa", bufs=1, space="PSUM"))

        ident = const.tile([128, 128], F32)
        make_identity(nc, ident[:])
        ident16 = const.tile([128, 128], F16)
        nc.vector.tensor_copy(ident16[:], ident[:])
        iota_i = const.tile([128, C * 128], I32)
        nc.gpsimd.iota(iota_i[:], pattern=[[0, C], [1, 128]], base=0, channel_multiplier=0)
        iota_f = const.tile([128, C * 128], F16)
        nc.vector.tensor_copy(iota_f[:], iota_i[:])
        zeros = const.tile([128, 128], F32)
        nc.vector.memset(zeros[:], 0.0)

        slabA = slabp.tile([128, NPAD], F16, tag="slabA")
        slabB = slabp.tile([128, NPAD], F16, tag="slabB")
        nc.sync.dma_start(slabA[:IN_DIM, :], t_xT[:])

        wt = []
        for i in range(3):
            w = wp.tile([128, 108], F16, tag=f"w{i}")
            nc.sync.dma_start(w[:], t_wext[i][:])
            wt.append(w)
        bt = []
        for i in range(3):
            b = wp.tile([128, DENSE], F32, tag=f"b{i}")
            nc.sync.dma_start(b[:], t_bt[i][:])
            bt.append(b)

        for _rep in range(repeat):
          for layer in range(3):
            slab_in = [slabA, slabB, slabA][layer]
            slab_out = [slabB, slabA, None][layer]
            kdim = IN_DIM if layer == 0 else DENSE

            # ---- dense phase: hext = [h | es | ed] for local rows ----
            for b in range(NBLK):
                pd = psum.tile([128, 108], F32, tag="pd", space="PSUM")
                nc.tensor.matmul(out=pd[:], lhsT=slab_in[:kdim, b * ROW:(b + 1) * ROW],
                                 rhs=wt[layer][:kdim, :], start=True, stop=True)
                hx = iop.tile([128, 108], F16, tag="hx")
                nc.scalar.activation(hx[:], pd[:], mybir.ActivationFunctionType.Copy)
                nc.sync.dma_start(d_hextloc[b * ROW:(b + 1) * ROW, 0:108], hx[:])
                nc.sync.dma_start(d_edtab[b * ROW:(b + 1) * ROW, 0:4], hx[:, 104:108])

            nc.gpsimd.collective_compute(
                "AllGather", mybir.AluOpType.bypass,
                replica_groups=[list(range(NC))],
                ins=[d_hextloc[:].opt()], outs=[d_hexfull[:].opt()])

            # ---- aggregation phase ----
            for b in range(NBLK):
                mi = iop.tile([128, C * 8], I16, tag="mi")
                nc.sync.dma_start(mi[:], t_midx[b])
                ei = iop.tile([128, C * 8], I16, tag="ei")
                nc.sync.dma_start(ei[:], t_edidx[b])
                dr = iop.tile([128, C], F16, tag="dr")
                nc.sync.dma_start(dr[:], t_dstrel[b])

                M = big.tile([128, C, 128], F16, tag="M")
                nc.gpsimd.dma_gather(
                    out_ap=M[:, 0:TPA, :], in_ap=d_hexfull[0:HALF, :], idxs_ap=mi[:, 0:TPA * 8],
                    num_idxs=NIA, num_idxs_reg=NIA, elem_size=CPAD, single_packet=False)
                nc.gpsimd.dma_gather(
                    out_ap=M[:, TPA:, :], in_ap=d_hexfull[HALF:, :], idxs_ap=mi[:, TPA * 8:],
                    num_idxs=NIB, num_idxs_reg=NIB, elem_size=CPAD, single_packet=False)
                ED = big.tile([128, C, 128], F16, tag="ED")
                nc.gpsimd.dma_gather(
                    out_ap=ED[:, 0:TPA, :], in_ap=d_edtab[:], idxs_ap=ei[:, 0:TPA * 8],
                    num_idxs=NIA, num_idxs_reg=NIA, elem_size=CPAD, single_packet=False)
                nc.gpsimd.dma_gather(
                    out_ap=ED[:, TPA:, :], in_ap=d_edtab[:], idxs_ap=ei[:, TPA * 8:],
                    num_idxs=NIB, num_idxs_reg=NIB, elem_size=CPAD, single_packet=False)

                S = big.tile([128, C, 128], F16, tag="S")
                nc.vector.tensor_tensor(
                    out=S[:], in0=dr[:].unsqueeze(2).to_broadcast([128, C, 128]),
                    in1=iota_f[:].rearrange("p (c k) -> p c k", k=128),
                    op=mybir.AluOpType.is_equal)

                al = sm.tile([128, C, 4], F16, tag="al")
                nc.vector.tensor_tensor(
                    out=al[:], in0=M[:, :, 100:104], in1=ED[:, :, 0:4],
                    op=mybir.AluOpType.add)
                lr = sm.tile([128, C, 4], F16, tag="lr")
                nc.vector.tensor_scalar_mul(lr[:], al[:], NEG)
                nc.vector.tensor_tensor(out=lr[:], in0=al[:], in1=lr[:], op=mybir.AluOpType.max)
                Mw = big.tile([128, C, 104], F16, tag="Mw")
                nc.scalar.activation(
                    Mw[:, :, 100:104], lr[:], mybir.ActivationFunctionType.Exp)
                nc.vector.tensor_tensor(
                    out=Mw[:, :, 0:100].rearrange("p c (h d) -> p c h d", h=4),
                    in0=M[:, :, 0:100].rearrange("p c (h d) -> p c h d", h=4),
                    in1=Mw[:, :, 100:104].unsqueeze(3).to_broadcast([128, C, 4, 25]),
                    op=mybir.AluOpType.mult)

                po = psa.tile([128, 104], F32, tag="po", space="PSUM")
                for t in range(C):
                    nc.tensor.matmul(out=po[:], lhsT=S[:, t, :], rhs=Mw[:, t, :],
                                     start=(t == 0), stop=(t == C - 1))

                rd = sm.tile([128, 4], F32, tag="rd")
                nc.vector.tensor_scalar_add(rd[:], po[:, 100:104], 1e-16)
                nc.vector.reciprocal(rd[:], rd[:])
                th = sm.tile([128, DENSE], F32, tag="th")
                nc.vector.tensor_tensor(
                    out=th[:].rearrange("p (h d) -> p h d", h=4),
                    in0=po[:, 0:100].rearrange("p (h d) -> p h d", h=4),
                    in1=rd[:].unsqueeze(2).to_broadcast([128, 4, 25]),
                    op=mybir.AluOpType.mult)
                nc.vector.tensor_tensor(out=th[:], in0=th[:], in1=bt[layer][:],
                                        op=mybir.AluOpType.add)
                th16 = sm.tile([128, DENSE], F16, tag="th16")
                nc.scalar.activation(th16[:], th[:], mybir.ActivationFunctionType.Relu)

                if layer < 2:
                    pt = psum.tile([DENSE, 128], F16, tag="pt", space="PSUM")
                    nc.tensor.transpose(out=pt[:], in_=th16[:], identity=ident16[:])
                    nc.scalar.activation(slab_out[0:DENSE, b * ROW:(b + 1) * ROW], pt[:],
                                         mybir.ActivationFunctionType.Copy)
                else:
                    tho = sm.tile([128, DENSE], F32, tag="tho")
                    nc.scalar.activation(tho[:], th[:], mybir.ActivationFunctionType.Relu)
                    nc.sync.dma_start(o_node[b * ROW:(b + 1) * ROW, :], tho[:])
                    nc.sync.dma_start(d_h3tab[b * ROW:(b + 1) * ROW, 0:DENSE], th16[:])

        # zero pad rows of h3tab (rows NPC.. get pooled via pad idx)
        z16 = const.tile([128, 128], F16)
        nc.vector.memset(z16[:], 0.0)
        nc.sync.dma_start(d_h3tab[NPC:NPC + 128, :], z16[:])
        nc.sync.dma_start(d_canvas[0:G + 1, :], zeros[:G + 1, 0:DENSE])

        # ---- pooling: per local graph max over its node rows ----
        gmat = const.tile([128, GLP], F32)
        for j in range(GLP):
            pi = iop.tile([128, PC * 8], I16, tag="pi")
            nc.sync.dma_start(pi[:], t_pidx[j])
            PG = big.tile([128, PC, 128], F16, tag="PG")
            nc.gpsimd.dma_gather(
                out_ap=PG[:], in_ap=d_h3tab[:], idxs_ap=pi[:],
                num_idxs=PC * 128, num_idxs_reg=PC * 128, elem_size=128,
                single_packet=False)
            gacc = sm.tile([128, 128], F32, tag="gacc")
            nc.vector.memset(gacc[:], 0.0)
            for c in range(PC):
                pp = psum.tile([128, 128], F16, tag="pp", space="PSUM")
                nc.tensor.transpose(out=pp[:], in_=PG[:, c, :], identity=ident16[:])
                nc.vector.tensor_tensor(out=gacc[:], in0=gacc[:], in1=pp[:],
                                        op=mybir.AluOpType.max)
            nc.vector.tensor_reduce(out=gmat[:, j:j + 1], in_=gacc[:],
                                    axis=mybir.AxisListType.X, op=mybir.AluOpType.max)

        gT = sm.tile([128, 128], F32, tag="gT")
        pgt = psum.tile([GLP, 128], F32, tag="pgt", space="PSUM")
        nc.tensor.transpose(out=pgt[:], in_=gmat[:], identity=ident[:])
        nc.scalar.activation(gT[:GLP, :], pgt[:], mybir.ActivationFunctionType.Copy)
        gidt = sm.tile([GLP, 1], I32, tag="gidt")
        nc.sync.dma_start(gidt[:], t_gids[:])
        nc.gpsimd.indirect_dma_start(
            out=d_canvas[:], out_offset=bass.IndirectOffsetOnAxis(ap=gidt[:, 0:1], axis=0),
            in_=gT[:GLP, 0:DENSE], in_offset=None)

        nc.gpsimd.collective_compute(
            "AllReduce", mybir.AluOpType.max,
            replica_groups=[list(range(NC))],
            ins=[d_canvas[0:G, :].opt()], outs=[d_gout[:].opt()])

        # ---- prototype head (replicated) ----
        ge = sm.tile([G, DENSE], F32, tag="ge")
        nc.sync.dma_start(ge[:], d_gout[:])
        nc.sync.dma_start(o_gemb[:], d_gout[:])
        gsq = sm.tile([G, DENSE], F32, tag="gsq")
        nc.scalar.activation(gsq[:], ge[:], mybir.ActivationFunctionType.Square)
        ng = sm.tile([G, 1], F32, tag="ng")
        nc.vector.tensor_reduce(out=ng[:], in_=gsq[:], axis=mybir.AxisListType.X,
                                op=mybir.AluOpType.add)
        pge = psum.tile([DENSE, G], F32, tag="pge", space="PSUM")
        nc.tensor.transpose(out=pge[:], in_=ge[:], identity=ident[:G, :G])
        geT = sm.tile([128, G], F32, tag="geT")
        nc.vector.memset(geT[:], 1.0)
        nc.scalar.activation(geT[0:DENSE, :], pge[:], mybir.ActivationFunctionType.Copy)

        prh = sm.tile([128, NUM_PROT], F32, tag="prh")
        nc.sync.dma_start(prh[:], t_prhs[:])
        pdist = psum.tile([G, NUM_PROT], F32, tag="pge", space="PSUM")
        nc.tensor.matmul(out=pdist[:], lhsT=geT[0:DENSE + 1, :], rhs=prh[0:DENSE + 1, :],
                         start=True, stop=True)
        dist = sm.tile([G, NUM_PROT], F32, tag="dist")
        nc.vector.tensor_tensor(out=dist[:], in0=pdist[:],
                                in1=ng[:].to_broadcast([G, NUM_PROT]),
                                op=mybir.AluOpType.add)
        nc.sync.dma_start(o_dist[:], dist[:])

        d1 = sm.tile([G, NUM_PROT], F32, tag="d1")
        nc.vector.tensor_scalar_add(d1[:], dist[:], 1.0)
        l1 = sm.tile([G, NUM_PROT], F32, tag="l1")
        nc.scalar.activation(l1[:], d1[:], mybir.ActivationFunctionType.Ln)
        d2 = sm.tile([G, NUM_PROT], F32, tag="d2")
        nc.vector.tensor_scalar_add(d2[:], dist[:], EPS)
        l2 = sm.tile([G, NUM_PROT], F32, tag="l2")
        nc.scalar.activation(l2[:], d2[:], mybir.ActivationFunctionType.Ln)
        sim = sm.tile([G, NUM_PROT], F32, tag="sim")
        nc.vector.tensor_tensor(out=sim[:], in0=l1[:], in1=l2[:],
                                op=mybir.AluOpType.subtract)

        psim = psum.tile([NUM_PROT, G], F32, tag="psim", space="PSUM")
        nc.tensor.transpose(out=psim[:], in_=sim[:], identity=ident[:G, :G])
        simT = sm.tile([NUM_PROT, G], F32, tag="simT")
        nc.scalar.activation(simT[:], psim[:], mybir.ActivationFunctionType.Copy)
        lwT = sm.tile([NUM_PROT, OUT_DIM], F32, tag="lwT")
        nc.sync.dma_start(lwT[:], t_lwT[:])
        plog = psum.tile([G, OUT_DIM], F32, tag="psim", space="PSUM")
        nc.tensor.matmul(out=plog[:], lhsT=simT[:], rhs=lwT[:], start=True, stop=True)
        lg = sm.tile([G, OUT_DIM], F32, tag="lg")
        nc.scalar.activation(lg[:], plog[:], mybir.ActivationFunctionType.Copy)
        nc.sync.dma_start(o_logits[:], lg[:])

        mx = sm.tile([G, 1], F32, tag="mx")
        nc.vector.tensor_reduce(out=mx[:], in_=lg[:], axis=mybir.AxisListType.X,
                                op=mybir.AluOpType.max)
        zz = sm.tile([G, OUT_DIM], F32, tag="zz")
        nc.vector.tensor_tensor(out=zz[:], in0=lg[:],
                                in1=mx[:].to_broadcast([G, OUT_DIM]),
                                op=mybir.AluOpType.subtract)
        ez = sm.tile([G, OUT_DIM], F32, tag="ez")
        nc.scalar.activation(ez[:], zz[:], mybir.ActivationFunctionType.Exp)
        se = sm.tile([G, 1], F32, tag="se")
        nc.vector.tensor_reduce(out=se[:], in_=ez[:], axis=mybir.AxisListType.X,
                                op=mybir.AluOpType.add)
        nc.vector.reciprocal(se[:], se[:])
        pr = sm.tile([G, OUT_DIM], F32, tag="pr")
        nc.vector.tensor_tensor(out=pr[:], in0=ez[:],
                                in1=se[:].to_broadcast([G, OUT_DIM]),
                                op=mybir.AluOpType.mult)
        nc.sync.dma_start(o_probs[:], pr[:])
        ctx.close()

    nc.compile()
    return nc


def _weight_prep(inputs):
    """Fold attention vectors into dense weights; build head constants."""
    wext, bts = [], []
    for i in range(3):
        W = np.asarray(inputs[f"W{i}"], np.float32)
        a_s = np.asarray(inputs[f"as{i}"], np.float32)
        a_d = np.asarray(inputs[f"ad{i}"], np.float32)
        d_in = W.shape[0]
        Asrc = np.zeros((DENSE, HEADS), np.float32)
        Adst = np.zeros((DENSE, HEADS), np.float32)
        for h in range(HEADS):
            Asrc[h * HID:(h + 1) * HID, h] = a_s[h]
            Adst[h * HID:(h + 1) * HID, h] = a_d[h]
        we = np.zeros((128, 108), np.float32)
        we[:d_in, 0:100] = W
        we[:d_in, 100:104] = W @ Asrc
        we[:d_in, 104:108] = W @ Adst
        wext.append(we.astype(np.float16))
        bts.append(np.tile(np.asarray(inputs[f"b{i}"], np.float32)[None, :], (128, 1)))
    protos = np.asarray(inputs["protos"], np.float32)
    prhs = np.zeros((128, NUM_PROT), np.float32)
    prhs[0:DENSE, :] = -2.0 * protos.T
    prhs[DENSE, :] = (protos ** 2).sum(axis=1)
    lwT = np.asarray(inputs["last_w"], np.float32).T.copy()
    return wext, bts, prhs, lwT


def kernel(**inputs):
    t0 = time.time()
    x = np.asarray(inputs["x"], np.float32)
    cores, TPA, TPB_, GLP, PC = preprocess(inputs["edge_index"], inputs["batch"])
    t1 = time.time()
    rep = int(os.environ.get("GAT_REPEAT", "1"))
    nc = build(TPA, TPB_, GLP, PC, repeat=rep)
    t2 = time.time()
    wext, bts, prhs, lwT = _weight_prep(inputs)

    in_maps = []
    for k in range(NC):
        xloc = np.zeros((NPAD, IN_DIM), np.float32)
        xloc[:NPC] = x[k * NPC:(k + 1) * NPC]
        im = dict(
            xT=np.ascontiguousarray(xloc.T).astype(np.float16),
            midx=cores[k]["midx"], edidx=cores[k]["edidx"], dstrel=cores[k]["dstrel"],
            pidx=cores[k]["pidx"], gids=cores[k]["gids"],
            prhs=prhs, lwT=lwT)
        for i in range(3):
            im[f"wext{i}"] = wext[i]
            im[f"bt{i}"] = bts[i]
        in_maps.append(im)

    t3 = time.time()
    res = run_bass_kernel_spmd(nc, in_maps, list(range(NC)))
    t4 = time.time()
    print(f"[kernel] prep={t1-t0:.1f}s build+sched+compile={t2-t1:.1f}s "
          f"inmaps={t3-t2:.1f}s run={t4-t3:.1f}s (repeat={rep})")
    node_emb = np.concatenate([res.results[k]["o_node"][:NPC] for k in range(NC)], axis=0)
    r0 = res.results[0]
    return (r0["o_logits"], r0["o_probs"], node_emb, r0["o_gemb"], r0["o_dist"])
